# revision 37
# baseline (speedup 1.0000x reference)
"""Trainium2 Bass kernel for nn_Attention_10771777978404 (sparse_attention).

Batch x head (tensor+data parallel) sharding over 8 NeuronCores:
  - SPMD: all cores run the same program; the data differs. Core ci handles
    batch ci//4 and head group ci%4 (4 of the 16 heads): its q/k/v projection
    column slices, RoPE, causal attention with the low-rank sigmoid gate, and
    a row-sharded wo partial for its batch. The host sums the 4 partials per
    batch.
  - the rank-32 adapter (gate) weights are replicated; both adapter rows are
    projected in one 64-wide-stationary pass. Gate tiles tanh((ak^T aq)/2)
    are computed per query block during the first head pass and reused by the
    second (tanh lives in the same ACT table as exp: sigmoid is applied as
    (1 + tanh)/2 with the /2 folded into the host-side wo scale).
  - softmax row sums come from a ones-stationary PE matmul; the reciprocal is
    broadcast across partitions with a tiny PE outer-product matmul.
  - causal diagonal-band tiles are trimmed: for band tile j only queries
    q >= 128*j within the 512-query block are computed, and the 0/1 triangle
    mask is applied to just the diagonal 128x128 sub-block.
  - attention processes the 4 heads in two 2-head passes (PSUM bank budget),
    software-pipelined: rowsum+AV matmuls run one tile behind the score
    matmuls so the exp/mask/gate chain latency hides under the PE stream;
    the output projection of each finished query block is queued and drained
    one chunk per tile stage of the following blocks.
  - SBUF is reused across phases: the gate tiles rotate into the dead
    wq/wk/wv regions, og into the dead x region, and the wo output staging
    into the dead RoPE-table regions.

Everything on-device is bf16 with fp32 PSUM accumulation.

self-contained: hardcodes the problem shapes; only needs `concourse` (on
PYTHONPATH in this container) + jax axon devices.
"""

import math
from dataclasses import dataclass

import numpy as np
import ml_dtypes

import concourse.tile as tile
from concourse import bacc
from concourse import mybir
from concourse import bass_utils

BF16 = mybir.dt.bfloat16
F32 = mybir.dt.float32
AF = mybir.ActivationFunctionType
ALU = mybir.AluOpType


@dataclass(frozen=True)
class Cfg:
    B: int = 2
    S: int = 2048
    DIM: int = 2048
    NH: int = 16
    HD: int = 128
    RANK: int = 32
    NCORES: int = 8
    QT: int = 512   # query block (free dim of score tiles)
    KT: int = 128   # key block (partition dim of score tiles)

    @property
    def CPB(self):
        return self.NCORES // self.B   # cores per batch

    @property
    def HLOC(self):
        return self.NH // self.CPB     # heads per core (4)

    @property
    def DH(self):
        return self.HLOC * self.HD     # per-core head-dim span (512)

    @property
    def KTILES(self):
        return self.DIM // 128  # contraction tiles for projections

    @property
    def QTN(self):
        return self.S // self.QT

    @property
    def DIAG(self):
        return self.QT // self.KT  # k-tiles per diagonal band


FULL = Cfg()


def build_nc(cfg: Cfg = FULL, *, marks=None, trim=True):
    c = cfg
    assert c.HD == 128 and c.KT == 128
    nc = bacc.Bacc("TRN2", target_bir_lowering=False, debug=False,
                   num_devices=c.NCORES)
    mark = (lambda tag: marks.append((tag, nc.next_id()))) if marks is not None \
        else (lambda tag: None)

    # ---- kernel I/O (per core: ONE batch, 4 heads) ----
    xT = nc.dram_tensor("xT", [c.DIM, c.S], BF16, kind="ExternalInput")
    wqT = nc.dram_tensor("wqT", [c.DIM, c.DH], BF16, kind="ExternalInput")
    wkT = nc.dram_tensor("wkT", [c.DIM, c.DH], BF16, kind="ExternalInput")
    wvT = nc.dram_tensor("wvT", [c.DIM, c.DH], BF16, kind="ExternalInput")
    # woc^T[d_local, j]: this core's head-rows of wo^T, pre-scaled by 0.5 on
    # the host (gate = (1+tanh)/2 is applied as (1+tanh) on device)
    wocT = nc.dram_tensor("wocT", [c.DH, c.DIM], BF16, kind="ExternalInput")
    waT = nc.dram_tensor("waT", [c.DIM, 2 * c.RANK], BF16, kind="ExternalInput")
    c2d = nc.dram_tensor("c2d", [c.HD, c.S], BF16, kind="ExternalInput")
    s2d = nc.dram_tensor("s2d", [c.HD, c.S], BF16, kind="ExternalInput")
    pswapd = nc.dram_tensor("pswapd", [c.HD, c.HD], BF16, kind="ExternalInput")
    # 0/1 lower-triangle pattern for the diagonal 128x128 block: tri[k, q]=1
    # iff q >= k (multiplicative form of the causal mask)
    trid = nc.dram_tensor("trid", [c.KT, c.KT], BF16, kind="ExternalInput")

    # partial output projection for this core's batch, transposed:
    # pout[j, t] (bf16; host sums the 4 partials per batch in f32)
    pout = nc.dram_tensor("pout", [c.DIM, c.S], BF16, kind="ExternalOutput")

    isqrt = 1.0 / math.sqrt(c.HD)
    NQC = c.DH // 128          # q/k head chunks per core (= HLOC = 4)

    from contextlib import ExitStack
    with ExitStack() as _ctx:
        tc = _ctx.enter_context(tile.TileContext(nc))
        cst = _ctx.enter_context(tc.tile_pool(name="const", bufs=1))
        xtp = _ctx.enter_context(tc.tile_pool(name="xt", bufs=1))
        qkp = _ctx.enter_context(tc.tile_pool(name="qk", bufs=1))
        vp = _ctx.enter_context(tc.tile_pool(name="vp", bufs=1))
        adp = _ctx.enter_context(tc.tile_pool(name="ap", bufs=1))
        rtp = _ctx.enter_context(tc.tile_pool(name="rope_t", bufs=1))
        # wq/wk/wv stationaries die after the projection phases; the per-qt
        # gate tiles rotate into their regions afterwards
        wwp = _ctx.enter_context(tc.tile_pool(name="wqkv", bufs=3))
        pge = _ctx.enter_context(tc.tile_pool(name="pge", bufs=4))
        nrm = _ctx.enter_context(tc.tile_pool(name="norm", bufs=1))
        # c2/s2 die after the fused-RoPE projections; wo out tiles rotate in
        rcp = _ctx.enter_context(tc.tile_pool(name="ropec", bufs=2))
        # PSUM: 8 banks total: pp(2) + psp(2) + pop(2) + prsp(2)
        pp = _ctx.enter_context(tc.tile_pool(name="pp", bufs=2, space="PSUM"))
        psp = _ctx.enter_context(tc.tile_pool(name="ps", bufs=2, space="PSUM"))
        pop = _ctx.enter_context(tc.tile_pool(name="po", bufs=2, space="PSUM"))
        prsp = _ctx.enter_context(tc.tile_pool(name="prs", bufs=2, space="PSUM"))
        if True:
            # ---- constants / weights ----
            wq_sb = wwp.tile([128, c.KTILES, c.DH], BF16, name="wq_sb", tag="wrot")
            wk_sb = wwp.tile([128, c.KTILES, c.DH], BF16, name="wk_sb", tag="wrot")
            wv_sb = wwp.tile([128, c.KTILES, c.DH], BF16, name="wv_sb", tag="wrot")
            woc_sb = cst.tile([128, NQC, c.DIM], BF16, name="woc_sb")
            wa_sb = cst.tile([128, c.KTILES, 2 * c.RANK], BF16, name="wa_sb")
            c2_sb = rcp.tile([128, c.S], BF16, name="c2_sb", tag="rc")
            s2_sb = rcp.tile([128, c.S], BF16, name="s2_sb", tag="rc")
            psw_sb = cst.tile([128, 128], BF16, name="psw_sb")
            tri_sb = cst.tile([128, 128], BF16, name="tri_sb")
            ones_sb = cst.tile([128, 1], BF16, name="ones_sb")
            onesf_sb = cst.tile([1, 128], F32, name="onesf_sb")

            # DMA order matters (FIFO queue): the adapter weights and the x
            # tiles go first so the adapter accumulation chain can start
            # streaming as x tiles land; the projection weights ride behind
            # them and arrive long before they're needed.
            nc.sync.dma_start(out=wa_sb, in_=waT.ap().rearrange("(t p) m -> p t m", p=128))

            xt_sb = xtp.tile([128, c.KTILES, c.S], BF16, name="xt_sb", tag="xt")
            xr = xT.ap().rearrange("(t p) n -> p t n", p=128)
            for kt in range(c.KTILES):
                # split across both HWDGE queues (SP + ACT): at kernel start
                # the ACT stream is empty, so both queues pull concurrently
                eng = nc.sync if kt % 2 == 0 else nc.scalar
                eng.dma_start(out=xt_sb[:, kt, :], in_=xr[:, kt, :])

            for w_sb, w_d in ((wq_sb, wqT), (wk_sb, wkT), (wv_sb, wvT)):
                wr = w_d.ap().rearrange("(t p) m -> p t m", p=128)
                for half in range(2):
                    h0 = half * (c.KTILES // 2)
                    nc.sync.dma_start(out=w_sb[:, h0:h0 + c.KTILES // 2, :],
                                      in_=wr[:, h0:h0 + c.KTILES // 2, :])
            nc.sync.dma_start(out=c2_sb, in_=c2d.ap())
            nc.sync.dma_start(out=s2_sb, in_=s2d.ap())
            nc.sync.dma_start(out=psw_sb, in_=pswapd.ap())
            nc.sync.dma_start(out=tri_sb, in_=trid.ap())
            wcr = wocT.ap().rearrange("(h p) j -> p h j", p=128)
            for h in range(NQC):
                nc.sync.dma_start(out=woc_sb[:, h, :], in_=wcr[:, h, :])
            nc.vector.memset(ones_sb, 1.0)
            nc.vector.memset(onesf_sb, 1.0)

            # ---- adapter projections: both rank-32 rows in one pass ----
            mark("adapters")
            aq_sb = adp.tile([32, c.S], BF16, name="aq_sb", tag="aq")
            ak_sb = adp.tile([32, c.S], BF16, name="ak_sb", tag="ak")
            for qt in range(c.QTN):
                qsl = slice(qt * c.QT, (qt + 1) * c.QT)
                psum = pp.tile([64, c.QT], F32, name="psum_a", tag="pp")
                for kt in range(c.KTILES):
                    nc.tensor.matmul(
                        psum[:, :],
                        wa_sb[:, kt, :],
                        xt_sb[:, kt, qsl],
                        start=(kt == 0), stop=(kt == c.KTILES - 1))
                nc.vector.tensor_copy(aq_sb[:, qsl], psum[0:32, :])
                nc.vector.tensor_copy(ak_sb[:, qsl], psum[32:64, :])

            # ---- q^T, k^T: [d, tok] per head chunk, RoPE fused in ----
            # swap(t) comes from a PE permutation matmul on the projection;
            # the RoPE element-wise ops ride on VE under the next
            # 16-matmul accumulation chain.
            mark("qkproj")
            q_sb = [qkp.tile([128, c.S], BF16, name=f"q{h}_sb", tag=f"q{h}")
                    for h in range(NQC)]
            k_sb = [qkp.tile([128, c.S], BF16, name=f"k{h}_sb", tag=f"k{h}")
                    for h in range(NQC)]
            for dst, w in ((q_sb, wq_sb), (k_sb, wk_sb)):
                for h in range(NQC):
                    for qt in range(c.QTN):
                        sl = slice(qt * c.QT, (qt + 1) * c.QT)
                        psum = pp.tile([128, c.QT], F32, name="psum_qk", tag="pp")
                        for kt in range(c.KTILES):
                            nc.tensor.matmul(
                                psum[:, :],
                                w[:, kt, h * 128:(h + 1) * 128],
                                xt_sb[:, kt, sl],
                                start=(kt == 0), stop=(kt == c.KTILES - 1))
                        nc.scalar.copy(dst[h][:, sl], psum[:, :])
                        # swap via PE permutation on the raw projection
                        pswp = psp.tile([128, c.QT], F32, name="pswp", tag="ps")
                        nc.tensor.matmul(pswp[:, :], psw_sb[:, :],
                                         dst[h][:, sl], start=True, stop=True)
                        m2 = rtp.tile([128, c.QT], BF16, name="rope_m2", tag="m2")
                        nc.vector.tensor_mul(m2[:, :], pswp[:, :], s2_sb[:, sl])
                        # in-place: dst = dst*c2 + swap(dst)*s2 (the swap
                        # matmul above reads dst before this overwrite)
                        nc.vector.tensor_mul(dst[h][:, sl], dst[h][:, sl],
                                             c2_sb[:, sl])
                        nc.vector.tensor_add(dst[h][:, sl], dst[h][:, sl],
                                             m2[:, :])

            # ---- v: [tok, d] natural; stationary = x^T tile ----
            mark("vproj")
            v_sb = vp.tile([128, c.S // 128, c.DH], BF16, name="v_sb", tag="v")
            for tt in range(c.S // 128):
                psum = pp.tile([128, c.DH], F32, name="psum_v", tag="pp")
                for kt in range(c.KTILES):
                    nc.tensor.matmul(
                        psum[:, :],
                        xt_sb[:, kt, tt * 128:(tt + 1) * 128],
                        wv_sb[:, kt, :],
                        start=(kt == 0), stop=(kt == c.KTILES - 1))
                nc.vector.tensor_copy(v_sb[:, tt, :], psum[:, :])

            # ---- attention: scores + rowsum + AV per 512-query block qt,
            # 4 heads in two 2-head passes (PSUM banks), diagonal-band tiles
            # trimmed to q >= 128*j ----
            mark("attn")
            # og rotates into the x buffer (x's last reader is the v
            # projection, which precedes all og writes)
            og_sb = xtp.tile([128, c.HLOC, c.S], BF16, name="og_sb", tag="xt")
            # output-projection work for a finished query block is queued and
            # drained a couple of chunks per tile stage of the NEXT block, so
            # the PE/ACT/VE streams stay smooth instead of bursting 16
            # matmuls + copies + DMAs at every qt boundary
            wo_pending = []

            def drain_wo(n):
                for _ in range(min(n, len(wo_pending))):
                    wo_pending.pop(0)()

            HP = 2  # heads per pass
            for qt in range(c.QTN):
                mark("attn_qt")
                nkt = c.DIAG * qt + c.DIAG  # causal k tiles
                qsl = slice(qt * c.QT, (qt + 1) * c.QT)

                def tile_geom(kt):
                    j = kt - c.DIAG * qt  # >= 0 on the diagonal band
                    q0 = c.KT * j if (trim and j > 0) else 0
                    return j, q0, c.QT - q0

                # gate tiles tanh((ak^T aq)/2) for this query block, computed
                # inline during the first head pass (an independent PE matmul
                # per tile stage), reused by the second pass
                gq_sb = wwp.tile([128, c.KTILES, c.QT], BF16, name="gq_sb",
                                 tag="wrot")

                for pi in range(c.HLOC // HP):
                    heads = range(pi * HP, (pi + 1) * HP)
                    po = {h: pop.tile([128, c.QT], F32, name=f"po{h}", tag="po")
                          for h in heads}
                    prs = {h: prsp.tile([1, c.QT], F32, name=f"prs{h}", tag="prs")
                           for h in heads}

                    # compute stage: score matmuls and the ACT/VE chain
                    # (exp/mask/gated-probs); PE-side consume stage
                    # (rowsum+AV) runs one tile behind so the chain latency
                    # hides under the next tile's matmuls.
                    staged = {}

                    def compute(kt, pi=pi):
                        ksl = slice(kt * c.KT, (kt + 1) * c.KT)
                        j, q0, qw = tile_geom(kt)
                        qsl_t = slice(qt * c.QT + q0, (qt + 1) * c.QT)
                        if pi == 0:
                            pga = psp.tile([128, c.QT], F32, name="pga", tag="ps")
                            nc.tensor.matmul(pga[:, :qw], ak_sb[:, ksl],
                                             aq_sb[:, qsl_t], start=True, stop=True)
                            nc.scalar.activation(gq_sb[:, kt, :qw], pga[:, :qw],
                                                 AF.Tanh, scale=0.5)
                        pergm = []
                        for h in heads:
                            ps = psp.tile([128, c.QT], F32, name="ps", tag="ps")
                            nc.tensor.matmul(ps[:, :qw], k_sb[h][:, ksl],
                                             q_sb[h][:, qsl_t], start=True, stop=True)
                            p_sb = pge.tile([128, c.QT], BF16, name="p_sb", tag="p")
                            nc.scalar.activation(p_sb[:, :qw], ps[:, :qw],
                                                 AF.Exp, scale=isqrt)
                            if j >= 0:
                                # causal 0/1 triangle on the diagonal
                                # 128-block (post-exp multiply, in place)
                                nc.vector.tensor_mul(p_sb[:, :c.KT],
                                                     p_sb[:, :c.KT], tri_sb[:, :])
                            # gated probs: (tanh + 1) * p (the /2 lives in woc)
                            pgm = pge.tile([128, c.QT], BF16, name="pgm", tag="pgm")
                            nc.vector.scalar_tensor_tensor(
                                pgm[:, :qw], gq_sb[:, kt, :qw], 1.0,
                                p_sb[:, :qw], op0=ALU.add, op1=ALU.mult)
                            pergm.append((h, p_sb, pgm))
                        staged[kt] = pergm

                    def consume(kt):
                        j, q0, qw = tile_geom(kt)
                        first = (kt == 0)
                        last = (kt == nkt - 1)
                        for h, p_sb, pgm in staged[kt]:
                            # pre-gate rowsum via ones-vector matmul
                            nc.tensor.matmul(prs[h][:, q0:], ones_sb[:, :],
                                             p_sb[:, :qw],
                                             start=first, stop=last,
                                             skip_group_check=True)
                            # out_h^T[d, q] += v[k,d].T @ p_gated[k,q]
                            nc.tensor.matmul(po[h][:, q0:],
                                             v_sb[:, kt, h * 128:(h + 1) * 128],
                                             pgm[:, :qw],
                                             start=first, stop=last,
                                             skip_group_check=True)
                        del staged[kt]

                    for kt in range(nkt):
                        compute(kt)
                        if kt > 0:
                            consume(kt - 1)
                        drain_wo(1)
                    consume(nkt - 1)

                    # normalize: og = po * (1/rowsum); the reciprocal row is
                    # broadcast across partitions with a PE outer product.
                    mark("attn_norm")
                    rrs = {}
                    for h in heads:
                        rr = nrm.tile([1, c.QT], F32, name="rr", tag="rr")
                        nc.vector.reciprocal_approx_fast(out=rr[:, :],
                                                         in_=prs[h][:, :])
                        rrs[h] = rr
                    for h in heads:
                        pbc = psp.tile([128, c.QT], F32, name="pbc", tag="ps")
                        nc.tensor.matmul(pbc[:, :], onesf_sb[:, :], rrs[h][:, :],
                                         start=True, stop=True)
                        rbc = nrm.tile([128, c.QT], BF16, name="rbc", tag="rbc")
                        nc.vector.tensor_copy(rbc[:, :], pbc[:, :])
                        nc.vector.tensor_mul(og_sb[:, h, qsl], po[h][:, :],
                                             rbc[:, :])

                # ---- output-projection partial for this query block,
                # queued and drained through the next block's tile stages
                mark("attn_wo")

                def emit_wo(ch, qt=qt, qsl=qsl):
                    pf = pp.tile([128, c.QT], F32, name="pf", tag="pp")
                    for h in range(c.HLOC):
                        nc.tensor.matmul(
                            pf[:, :],
                            woc_sb[:, h, ch * 128:(ch + 1) * 128],
                            og_sb[:, h, qsl],
                            start=(h == 0), stop=(h == c.HLOC - 1))
                    f_sb = rcp.tile([128, c.QT], BF16, name="f_sb", tag="rc")
                    nc.vector.tensor_copy(f_sb[:, :], pf[:, :])
                    nc.sync.dma_start(
                        out=pout.ap()[ch * 128:(ch + 1) * 128,
                                      qt * c.QT:(qt + 1) * c.QT],
                        in_=f_sb[:, :])

                for ch in range(c.DIM // 128):
                    wo_pending.append(lambda ch=ch: emit_wo(ch))
            # flush the last query block's output projection
            drain_wo(len(wo_pending))

    nc.compile()
    return nc


def make_core_inputs(inputs: dict, cfg: Cfg = FULL):
    """Host-side sharding: returns in_maps (one dict per core). Core ci
    handles batch ci // CPB with head group ci % CPB."""
    c = cfg
    bf16 = ml_dtypes.bfloat16
    x = np.asarray(inputs["x"])
    mask = np.asarray(inputs["mask"])
    fc = np.asarray(inputs["freqs_cos"])
    fs = np.asarray(inputs["freqs_sin"])
    wq, wk, wv, wo = (np.asarray(inputs[k]) for k in ("wq", "wk", "wv", "wo"))
    wa_q, wa_k = np.asarray(inputs["wa_q"]), np.asarray(inputs["wa_k"])

    xT = [np.ascontiguousarray(x[b].T).astype(bf16) for b in range(c.B)]
    waT = np.ascontiguousarray(np.concatenate([wa_q, wa_k], axis=0).T).astype(bf16)

    # rope tables in [d, tok] layout
    c2 = np.empty((c.HD, c.S), np.float32)
    s2 = np.empty((c.HD, c.S), np.float32)
    c2[0::2] = fc.T
    c2[1::2] = fc.T
    s2[0::2] = -fs.T
    s2[1::2] = fs.T
    c2 = c2.astype(bf16)
    s2 = s2.astype(bf16)

    psw = np.zeros((c.HD, c.HD), np.float32)
    idx = np.arange(c.HD)
    psw[idx, idx ^ 1] = 1.0
    psw = psw.astype(bf16)

    # diagonal 128x128 triangle pattern tri[k, q] = 1 iff query q can see
    # key k, extracted from the additive input mask (0 = visible)
    tri = (mask[0, 0, 0:c.KT, 0:c.KT].T == 0.0).astype(np.float32).astype(bf16)

    # per-head-group weight slices (shared between the two batches)
    wslices = []
    for hg in range(c.CPB):
        rows = slice(hg * c.DH, (hg + 1) * c.DH)
        wslices.append({
            "wqT": np.ascontiguousarray(wq[rows].T).astype(bf16),
            "wkT": np.ascontiguousarray(wk[rows].T).astype(bf16),
            "wvT": np.ascontiguousarray(wv[rows].T).astype(bf16),
            # 0.5x: the device applies the gate as (1 + tanh(a/2))
            "wocT": np.ascontiguousarray(0.5 * wo[:, rows].T).astype(bf16),
        })

    in_maps = []
    for ci in range(c.NCORES):
        bi, hg = ci // c.CPB, ci % c.CPB
        in_maps.append({
            "xT": xT[bi],
            **wslices[hg],
            "waT": waT,
            "c2d": c2,
            "s2d": s2,
            "pswapd": psw,
            "trid": tri,
        })
    return in_maps


def assemble_output(results, cfg: Cfg = FULL) -> np.ndarray:
    c = cfg
    out = np.empty((c.B, c.S, c.DIM), np.float32)
    for bi in range(c.B):
        total = np.zeros((c.DIM, c.S), np.float32)
        for hg in range(c.CPB):
            total += np.asarray(results[bi * c.CPB + hg]["pout"]).astype(np.float32)
        out[bi] = total.T
    return out


_NC_CACHE = {}


def run(nc, in_maps, trace=False, cfg: Cfg = FULL, **kw):
    return bass_utils.run_bass_kernel_spmd(
        nc, in_maps, core_ids=list(range(cfg.NCORES)), trace=trace, **kw)


def kernel(**inputs) -> np.ndarray:
    cfg = FULL
    if cfg not in _NC_CACHE:
        _NC_CACHE[cfg] = build_nc(cfg)
    nc = _NC_CACHE[cfg]
    in_maps = make_core_inputs(inputs, cfg)
    res = run(nc, in_maps, cfg=cfg)
    return assemble_output(res.results, cfg)


if __name__ == "__main__":
    nc = build_nc(FULL)
    print("built ok")


# revision 38
# speedup vs baseline: 1.1488x; 1.1488x over previous
"""Trainium2 Bass kernel for nn_Attention_10771777978404 (sparse_attention).

Batch x head (tensor+data parallel) sharding over 8 NeuronCores:
  - SPMD: all cores run the same program; the data differs. Core ci handles
    batch ci//4 and head group ci%4 (4 of the 16 heads): its q/k/v projection
    column slices, RoPE, causal attention with the low-rank sigmoid gate, and
    a row-sharded wo partial for its batch. The host sums the 4 partials per
    batch.
  - the rank-32 adapter (gate) weights are replicated; both adapter rows are
    projected in one 64-wide-stationary pass. Gate tiles tanh((ak^T aq)/2)
    are computed per query block during the first head pass and reused by the
    second (tanh lives in the same ACT table as exp: sigmoid is applied as
    (1 + tanh)/2 with the /2 folded into the host-side wo scale).
  - softmax row sums come from a ones-stationary PE matmul; the reciprocal is
    broadcast across partitions with a tiny PE outer-product matmul.
  - causal diagonal-band tiles are trimmed: for band tile j only queries
    q >= 128*j within the 512-query block are computed, and the 0/1 triangle
    mask is applied to just the diagonal 128x128 sub-block.
  - attention processes the 4 heads in two 2-head passes (PSUM bank budget),
    software-pipelined: rowsum+AV matmuls run one tile behind the score
    matmuls so the exp/mask/gate chain latency hides under the PE stream;
    the output projection of each finished query block is queued and drained
    one chunk per tile stage of the following blocks.
  - SBUF is reused across phases: the gate tiles rotate into the dead
    wq/wk/wv regions, og into the dead x region, and the wo output staging
    into the dead RoPE-table regions.

Everything on-device is bf16 with fp32 PSUM accumulation.

self-contained: hardcodes the problem shapes; only needs `concourse` (on
PYTHONPATH in this container) + jax axon devices.
"""

import math
from dataclasses import dataclass

import numpy as np
import ml_dtypes

import concourse.tile as tile
from concourse import bacc
from concourse import mybir
from concourse import bass_utils

BF16 = mybir.dt.bfloat16
F32 = mybir.dt.float32
AF = mybir.ActivationFunctionType
ALU = mybir.AluOpType


@dataclass(frozen=True)
class Cfg:
    B: int = 2
    S: int = 2048
    DIM: int = 2048
    NH: int = 16
    HD: int = 128
    RANK: int = 32
    NCORES: int = 8
    QT: int = 512   # query block (free dim of score tiles)
    KT: int = 128   # key block (partition dim of score tiles)

    @property
    def CPB(self):
        return self.NCORES // self.B   # cores per batch

    @property
    def HLOC(self):
        return self.NH // self.CPB     # heads per core (4)

    @property
    def DH(self):
        return self.HLOC * self.HD     # per-core head-dim span (512)

    @property
    def KTILES(self):
        return self.DIM // 128  # contraction tiles for projections

    @property
    def QTN(self):
        return self.S // self.QT

    @property
    def DIAG(self):
        return self.QT // self.KT  # k-tiles per diagonal band


FULL = Cfg()


def build_nc(cfg: Cfg = FULL, *, marks=None, trim=True):
    c = cfg
    assert c.HD == 128 and c.KT == 128
    nc = bacc.Bacc("TRN2", target_bir_lowering=False, debug=False,
                   num_devices=c.NCORES)
    mark = (lambda tag: marks.append((tag, nc.next_id()))) if marks is not None \
        else (lambda tag: None)

    # ---- kernel I/O (per core: ONE batch, 4 heads) ----
    xT = nc.dram_tensor("xT", [c.DIM, c.S], BF16, kind="ExternalInput")
    wqT = nc.dram_tensor("wqT", [c.DIM, c.DH], BF16, kind="ExternalInput")
    wkT = nc.dram_tensor("wkT", [c.DIM, c.DH], BF16, kind="ExternalInput")
    wvT = nc.dram_tensor("wvT", [c.DIM, c.DH], BF16, kind="ExternalInput")
    # woc^T[d_local, j]: this core's head-rows of wo^T, pre-scaled by 0.5 on
    # the host (gate = (1+tanh)/2 is applied as (1+tanh) on device)
    wocT = nc.dram_tensor("wocT", [c.DH, c.DIM], BF16, kind="ExternalInput")
    waT = nc.dram_tensor("waT", [c.DIM, 2 * c.RANK], BF16, kind="ExternalInput")
    c2d = nc.dram_tensor("c2d", [c.HD, c.S], BF16, kind="ExternalInput")
    s2d = nc.dram_tensor("s2d", [c.HD, c.S], BF16, kind="ExternalInput")
    pswapd = nc.dram_tensor("pswapd", [c.HD, c.HD], BF16, kind="ExternalInput")
    # 0/1 lower-triangle pattern for the diagonal 128x128 block: tri[k, q]=1
    # iff q >= k (multiplicative form of the causal mask)
    trid = nc.dram_tensor("trid", [c.KT, c.KT], BF16, kind="ExternalInput")

    # partial output projection for this core's batch, transposed:
    # pout[j, t] (bf16; host sums the 4 partials per batch in f32)
    pout = nc.dram_tensor("pout", [c.DIM, c.S], BF16, kind="ExternalOutput")

    isqrt = 1.0 / math.sqrt(c.HD)
    NQC = c.DH // 128          # q/k head chunks per core (= HLOC = 4)

    from contextlib import ExitStack
    with ExitStack() as _ctx:
        tc = _ctx.enter_context(tile.TileContext(nc))
        cst = _ctx.enter_context(tc.tile_pool(name="const", bufs=1))
        xtp = _ctx.enter_context(tc.tile_pool(name="xt", bufs=1))
        qkp = _ctx.enter_context(tc.tile_pool(name="qk", bufs=1))
        vp = _ctx.enter_context(tc.tile_pool(name="vp", bufs=1))
        adp = _ctx.enter_context(tc.tile_pool(name="ap", bufs=1))
        rtp = _ctx.enter_context(tc.tile_pool(name="rope_t", bufs=1))
        # wq/wk/wv stationaries die after the projection phases; the per-qt
        # gate tiles rotate into their regions afterwards
        wwp = _ctx.enter_context(tc.tile_pool(name="wqkv", bufs=3))
        pge = _ctx.enter_context(tc.tile_pool(name="pge", bufs=4))
        nrm = _ctx.enter_context(tc.tile_pool(name="norm", bufs=1))
        # c2/s2 die after the fused-RoPE projections; wo out tiles rotate in
        rcp = _ctx.enter_context(tc.tile_pool(name="ropec", bufs=2))
        # PSUM: 8 banks total: pp(2) + psp(2) + pop(2) + prsp(2)
        pp = _ctx.enter_context(tc.tile_pool(name="pp", bufs=2, space="PSUM"))
        psp = _ctx.enter_context(tc.tile_pool(name="ps", bufs=2, space="PSUM"))
        pop = _ctx.enter_context(tc.tile_pool(name="po", bufs=2, space="PSUM"))
        prsp = _ctx.enter_context(tc.tile_pool(name="prs", bufs=2, space="PSUM"))
        if True:
            # ---- constants / weights ----
            wq_sb = wwp.tile([128, c.KTILES, c.DH], BF16, name="wq_sb", tag="wrot")
            wk_sb = wwp.tile([128, c.KTILES, c.DH], BF16, name="wk_sb", tag="wrot")
            wv_sb = wwp.tile([128, c.KTILES, c.DH], BF16, name="wv_sb", tag="wrot")
            woc_sb = cst.tile([128, NQC, c.DIM], BF16, name="woc_sb")
            wa_sb = cst.tile([128, c.KTILES, 2 * c.RANK], BF16, name="wa_sb")
            c2_sb = rcp.tile([128, c.S], BF16, name="c2_sb", tag="rc")
            s2_sb = rcp.tile([128, c.S], BF16, name="s2_sb", tag="rc")
            psw_sb = cst.tile([128, 128], BF16, name="psw_sb")
            tri_sb = cst.tile([128, 128], BF16, name="tri_sb")
            ones_sb = cst.tile([128, 1], BF16, name="ones_sb")
            onesf_sb = cst.tile([1, 128], F32, name="onesf_sb")

            # DMA order matters (FIFO queue): the adapter weights and the x
            # tiles go first so the adapter accumulation chain can start
            # streaming as x tiles land; the projection weights ride behind
            # them and arrive long before they're needed.
            nc.sync.dma_start(out=wa_sb, in_=waT.ap().rearrange("(t p) m -> p t m", p=128))

            xt_sb = xtp.tile([128, c.KTILES, c.S], BF16, name="xt_sb", tag="xt")
            xr = xT.ap().rearrange("(t p) n -> p t n", p=128)
            for kt in range(c.KTILES):
                # split across both HWDGE queues (SP + ACT): at kernel start
                # the ACT stream is empty, so both queues pull concurrently
                eng = nc.sync if kt % 2 == 0 else nc.scalar
                eng.dma_start(out=xt_sb[:, kt, :], in_=xr[:, kt, :])

            for w_sb, w_d in ((wq_sb, wqT), (wk_sb, wkT), (wv_sb, wvT)):
                wr = w_d.ap().rearrange("(t p) m -> p t m", p=128)
                for half in range(2):
                    h0 = half * (c.KTILES // 2)
                    nc.sync.dma_start(out=w_sb[:, h0:h0 + c.KTILES // 2, :],
                                      in_=wr[:, h0:h0 + c.KTILES // 2, :])
            nc.sync.dma_start(out=c2_sb, in_=c2d.ap())
            nc.sync.dma_start(out=s2_sb, in_=s2d.ap())
            nc.sync.dma_start(out=psw_sb, in_=pswapd.ap())
            nc.sync.dma_start(out=tri_sb, in_=trid.ap())
            wcr = wocT.ap().rearrange("(h p) j -> p h j", p=128)
            for h in range(NQC):
                nc.sync.dma_start(out=woc_sb[:, h, :], in_=wcr[:, h, :])
            nc.vector.memset(ones_sb, 1.0)
            nc.vector.memset(onesf_sb, 1.0)

            # ---- adapter projections: both rank-32 rows in one pass ----
            mark("adapters")
            aq_sb = adp.tile([32, c.S], BF16, name="aq_sb", tag="aq")
            ak_sb = adp.tile([32, c.S], BF16, name="ak_sb", tag="ak")
            for qt in range(c.QTN):
                qsl = slice(qt * c.QT, (qt + 1) * c.QT)
                psum = pp.tile([64, c.QT], F32, name="psum_a", tag="pp")
                for kt in range(c.KTILES):
                    nc.tensor.matmul(
                        psum[:, :],
                        wa_sb[:, kt, :],
                        xt_sb[:, kt, qsl],
                        start=(kt == 0), stop=(kt == c.KTILES - 1))
                nc.vector.tensor_copy(aq_sb[:, qsl], psum[0:32, :])
                nc.vector.tensor_copy(ak_sb[:, qsl], psum[32:64, :])

            # ---- q^T, k^T: [d, tok] per head chunk, RoPE fused in ----
            # swap(t) comes from a PE permutation matmul on the projection;
            # the RoPE element-wise ops ride on VE under the next
            # 16-matmul accumulation chain.
            mark("qkproj")
            q_sb = [qkp.tile([128, c.S], BF16, name=f"q{h}_sb", tag=f"q{h}")
                    for h in range(NQC)]
            k_sb = [qkp.tile([128, c.S], BF16, name=f"k{h}_sb", tag=f"k{h}")
                    for h in range(NQC)]
            for dst, w in ((q_sb, wq_sb), (k_sb, wk_sb)):
                for h in range(NQC):
                    for qt in range(c.QTN):
                        sl = slice(qt * c.QT, (qt + 1) * c.QT)
                        psum = pp.tile([128, c.QT], F32, name="psum_qk", tag="pp")
                        for kt in range(c.KTILES):
                            nc.tensor.matmul(
                                psum[:, :],
                                w[:, kt, h * 128:(h + 1) * 128],
                                xt_sb[:, kt, sl],
                                start=(kt == 0), stop=(kt == c.KTILES - 1))
                        nc.scalar.copy(dst[h][:, sl], psum[:, :])
                        # swap via PE permutation on the raw projection
                        pswp = psp.tile([128, c.QT], F32, name="pswp", tag="ps")
                        nc.tensor.matmul(pswp[:, :], psw_sb[:, :],
                                         dst[h][:, sl], start=True, stop=True)
                        m2 = rtp.tile([128, c.QT], BF16, name="rope_m2", tag="m2")
                        nc.vector.tensor_mul(m2[:, :], pswp[:, :], s2_sb[:, sl])
                        # in-place: dst = dst*c2 + swap(dst)*s2 (the swap
                        # matmul above reads dst before this overwrite)
                        nc.vector.tensor_mul(dst[h][:, sl], dst[h][:, sl],
                                             c2_sb[:, sl])
                        nc.vector.tensor_add(dst[h][:, sl], dst[h][:, sl],
                                             m2[:, :])

            # ---- v: [tok, d] natural; stationary = x^T tile ----
            mark("vproj")
            v_sb = vp.tile([128, c.S // 128, c.DH], BF16, name="v_sb", tag="v")
            for tt in range(c.S // 128):
                psum = pp.tile([128, c.DH], F32, name="psum_v", tag="pp")
                for kt in range(c.KTILES):
                    nc.tensor.matmul(
                        psum[:, :],
                        xt_sb[:, kt, tt * 128:(tt + 1) * 128],
                        wv_sb[:, kt, :],
                        start=(kt == 0), stop=(kt == c.KTILES - 1))
                nc.vector.tensor_copy(v_sb[:, tt, :], psum[:, :])

            # ---- attention: scores + rowsum + AV per 512-query block qt,
            # 4 heads in two 2-head passes (PSUM banks), diagonal-band tiles
            # trimmed to q >= 128*j ----
            mark("attn")
            # og rotates into the x buffer (x's last reader is the v
            # projection, which precedes all og writes)
            og_sb = xtp.tile([128, c.HLOC, c.S], BF16, name="og_sb", tag="xt")
            # output-projection work for a finished query block is queued and
            # drained a couple of chunks per tile stage of the NEXT block, so
            # the PE/ACT/VE streams stay smooth instead of bursting 16
            # matmuls + copies + DMAs at every qt boundary
            wo_pending = []

            def drain_wo(n):
                for _ in range(min(n, len(wo_pending))):
                    wo_pending.pop(0)()

            HP = 2  # heads per pass
            for qt in range(c.QTN):
                mark("attn_qt")
                nkt = c.DIAG * qt + c.DIAG  # causal k tiles
                qsl = slice(qt * c.QT, (qt + 1) * c.QT)

                def tile_geom(kt):
                    j = kt - c.DIAG * qt  # >= 0 on the diagonal band
                    q0 = c.KT * j if (trim and j > 0) else 0
                    return j, q0, c.QT - q0

                # gate tiles tanh((ak^T aq)/2) for this query block, computed
                # inline during the first head pass (an independent PE matmul
                # per tile stage), reused by the second pass
                gq_sb = wwp.tile([128, c.KTILES, c.QT], BF16, name="gq_sb",
                                 tag="wrot")

                for pi in range(c.HLOC // HP):
                    heads = range(pi * HP, (pi + 1) * HP)
                    po = {h: pop.tile([128, c.QT], F32, name=f"po{h}", tag="po")
                          for h in heads}
                    prs = {h: prsp.tile([1, c.QT], F32, name=f"prs{h}", tag="prs")
                           for h in heads}

                    # compute stage: score matmuls and the ACT/VE chain
                    # (exp/mask/gated-probs); PE-side consume stage
                    # (rowsum+AV) runs one tile behind so the chain latency
                    # hides under the next tile's matmuls.
                    staged = {}

                    def compute(kt, pi=pi):
                        ksl = slice(kt * c.KT, (kt + 1) * c.KT)
                        j, q0, qw = tile_geom(kt)
                        qsl_t = slice(qt * c.QT + q0, (qt + 1) * c.QT)
                        if pi == 0:
                            pga = pp.tile([128, c.QT], F32, name="pga", tag="pp")
                            nc.tensor.matmul(pga[:, :qw], ak_sb[:, ksl],
                                             aq_sb[:, qsl_t], start=True, stop=True)
                            nc.scalar.activation(gq_sb[:, kt, :qw], pga[:, :qw],
                                                 AF.Tanh, scale=0.5)
                        pergm = []
                        for h in heads:
                            ps = psp.tile([128, c.QT], F32, name="ps", tag="ps")
                            nc.tensor.matmul(ps[:, :qw], k_sb[h][:, ksl],
                                             q_sb[h][:, qsl_t], start=True, stop=True)
                            p_sb = pge.tile([128, c.QT], BF16, name="p_sb", tag="p")
                            nc.scalar.activation(p_sb[:, :qw], ps[:, :qw],
                                                 AF.Exp, scale=isqrt)
                            if j >= 0:
                                # causal 0/1 triangle on the diagonal
                                # 128-block (post-exp multiply, in place)
                                nc.vector.tensor_mul(p_sb[:, :c.KT],
                                                     p_sb[:, :c.KT], tri_sb[:, :])
                            # gated probs: (tanh + 1) * p (the /2 lives in woc)
                            pgm = pge.tile([128, c.QT], BF16, name="pgm", tag="pgm")
                            nc.vector.scalar_tensor_tensor(
                                pgm[:, :qw], gq_sb[:, kt, :qw], 1.0,
                                p_sb[:, :qw], op0=ALU.add, op1=ALU.mult)
                            pergm.append((h, p_sb, pgm))
                        staged[kt] = pergm

                    def consume(kt):
                        j, q0, qw = tile_geom(kt)
                        first = (kt == 0)
                        last = (kt == nkt - 1)
                        for h, p_sb, pgm in staged[kt]:
                            # pre-gate rowsum via ones-vector matmul
                            nc.tensor.matmul(prs[h][:, q0:], ones_sb[:, :],
                                             p_sb[:, :qw],
                                             start=first, stop=last,
                                             skip_group_check=True)
                            # out_h^T[d, q] += v[k,d].T @ p_gated[k,q]
                            nc.tensor.matmul(po[h][:, q0:],
                                             v_sb[:, kt, h * 128:(h + 1) * 128],
                                             pgm[:, :qw],
                                             start=first, stop=last,
                                             skip_group_check=True)
                        del staged[kt]

                    for kt in range(nkt):
                        compute(kt)
                        if kt > 0:
                            consume(kt - 1)
                        drain_wo(1)
                    consume(nkt - 1)

                    # normalize: og = po * (1/rowsum); the reciprocal row is
                    # broadcast across partitions with a PE outer product.
                    mark("attn_norm")
                    rrs = {}
                    for h in heads:
                        rr = nrm.tile([1, c.QT], F32, name="rr", tag="rr")
                        nc.vector.reciprocal_approx_fast(out=rr[:, :],
                                                         in_=prs[h][:, :])
                        rrs[h] = rr
                    for h in heads:
                        pbc = psp.tile([128, c.QT], F32, name="pbc", tag="ps")
                        nc.tensor.matmul(pbc[:, :], onesf_sb[:, :], rrs[h][:, :],
                                         start=True, stop=True)
                        rbc = nrm.tile([128, c.QT], BF16, name="rbc", tag="rbc")
                        nc.vector.tensor_copy(rbc[:, :], pbc[:, :])
                        nc.vector.tensor_mul(og_sb[:, h, qsl], po[h][:, :],
                                             rbc[:, :])

                # ---- output-projection partial for this query block,
                # queued and drained through the next block's tile stages
                mark("attn_wo")

                def emit_wo(ch, qt=qt, qsl=qsl):
                    pf = pp.tile([128, c.QT], F32, name="pf", tag="pp")
                    for h in range(c.HLOC):
                        nc.tensor.matmul(
                            pf[:, :],
                            woc_sb[:, h, ch * 128:(ch + 1) * 128],
                            og_sb[:, h, qsl],
                            start=(h == 0), stop=(h == c.HLOC - 1))
                    f_sb = rcp.tile([128, c.QT], BF16, name="f_sb", tag="rc")
                    nc.vector.tensor_copy(f_sb[:, :], pf[:, :])
                    nc.sync.dma_start(
                        out=pout.ap()[ch * 128:(ch + 1) * 128,
                                      qt * c.QT:(qt + 1) * c.QT],
                        in_=f_sb[:, :])

                for ch in range(c.DIM // 128):
                    wo_pending.append(lambda ch=ch: emit_wo(ch))
            # flush the last query block's output projection
            drain_wo(len(wo_pending))

    nc.compile()
    return nc


def make_core_inputs(inputs: dict, cfg: Cfg = FULL):
    """Host-side sharding: returns in_maps (one dict per core). Core ci
    handles batch ci // CPB with head group ci % CPB."""
    c = cfg
    bf16 = ml_dtypes.bfloat16
    x = np.asarray(inputs["x"])
    mask = np.asarray(inputs["mask"])
    fc = np.asarray(inputs["freqs_cos"])
    fs = np.asarray(inputs["freqs_sin"])
    wq, wk, wv, wo = (np.asarray(inputs[k]) for k in ("wq", "wk", "wv", "wo"))
    wa_q, wa_k = np.asarray(inputs["wa_q"]), np.asarray(inputs["wa_k"])

    xT = [np.ascontiguousarray(x[b].T).astype(bf16) for b in range(c.B)]
    waT = np.ascontiguousarray(np.concatenate([wa_q, wa_k], axis=0).T).astype(bf16)

    # rope tables in [d, tok] layout
    c2 = np.empty((c.HD, c.S), np.float32)
    s2 = np.empty((c.HD, c.S), np.float32)
    c2[0::2] = fc.T
    c2[1::2] = fc.T
    s2[0::2] = -fs.T
    s2[1::2] = fs.T
    c2 = c2.astype(bf16)
    s2 = s2.astype(bf16)

    psw = np.zeros((c.HD, c.HD), np.float32)
    idx = np.arange(c.HD)
    psw[idx, idx ^ 1] = 1.0
    psw = psw.astype(bf16)

    # diagonal 128x128 triangle pattern tri[k, q] = 1 iff query q can see
    # key k, extracted from the additive input mask (0 = visible)
    tri = (mask[0, 0, 0:c.KT, 0:c.KT].T == 0.0).astype(np.float32).astype(bf16)

    # per-head-group weight slices (shared between the two batches)
    wslices = []
    for hg in range(c.CPB):
        rows = slice(hg * c.DH, (hg + 1) * c.DH)
        wslices.append({
            "wqT": np.ascontiguousarray(wq[rows].T).astype(bf16),
            "wkT": np.ascontiguousarray(wk[rows].T).astype(bf16),
            "wvT": np.ascontiguousarray(wv[rows].T).astype(bf16),
            # 0.5x: the device applies the gate as (1 + tanh(a/2))
            "wocT": np.ascontiguousarray(0.5 * wo[:, rows].T).astype(bf16),
        })

    in_maps = []
    for ci in range(c.NCORES):
        bi, hg = ci // c.CPB, ci % c.CPB
        in_maps.append({
            "xT": xT[bi],
            **wslices[hg],
            "waT": waT,
            "c2d": c2,
            "s2d": s2,
            "pswapd": psw,
            "trid": tri,
        })
    return in_maps


def assemble_output(results, cfg: Cfg = FULL) -> np.ndarray:
    c = cfg
    out = np.empty((c.B, c.S, c.DIM), np.float32)
    for bi in range(c.B):
        total = np.zeros((c.DIM, c.S), np.float32)
        for hg in range(c.CPB):
            total += np.asarray(results[bi * c.CPB + hg]["pout"]).astype(np.float32)
        out[bi] = total.T
    return out


_NC_CACHE = {}


def run(nc, in_maps, trace=False, cfg: Cfg = FULL, **kw):
    return bass_utils.run_bass_kernel_spmd(
        nc, in_maps, core_ids=list(range(cfg.NCORES)), trace=trace, **kw)


def kernel(**inputs) -> np.ndarray:
    cfg = FULL
    if cfg not in _NC_CACHE:
        _NC_CACHE[cfg] = build_nc(cfg)
    nc = _NC_CACHE[cfg]
    in_maps = make_core_inputs(inputs, cfg)
    res = run(nc, in_maps, cfg=cfg)
    return assemble_output(res.results, cfg)


if __name__ == "__main__":
    nc = build_nc(FULL)
    print("built ok")
